# revision 7
# baseline (speedup 1.0000x reference)
"""Trainium2 Bass kernel for nn_MultiHeadSelfAttention_88725434400988.

Self-contained: accepts FULL inputs, shards batch B=256 over 8 NeuronCores
(32 per core), runs one SPMD Bass program, gathers the FULL output.

Per-core algorithm (B_CORE=32, S=8, F=32, E=64, A=64, NH=2):
  - Hs, Wq, Wk, Wv, Wres cast to fp16 on host (PE matmuls run 1 cyc/row,
    fp32 PSUM accumulation; end-to-end error vs fp32 reference ~3e-3 absmax).
  - QK projection: lhsT = 128-col tiles of W, rhs = Hs^T (via transposing
    DMA); psum (128=(jh,a), 256=(b,s)).
  - Working order for attention rows/cols: p = jh*128 + sp*32 + f where the
    original index is sp*64 + 2f + jh (jh == partition half of psum).
    q/k gathered into qt/kt (64=a, b, nh, 256) fp16; the jh=1 psum half is
    staged and moved across partitions with one SBUF->SBUF DMA.
  - v: per (b,nh) transposing DMA of Hs rows -> lhsT (e, 128 rows);
    out (128 rows=(sp,f), 128=(h,a)) -> v_all (128=sigma tile, bn, 2, 64).
  - Z = qt_chunk.T @ kt (per jh chunk); exp on ScalarE with fused accum_out
    denominators (fp32; max |logit| ~49 so no max subtraction, fp16 would
    overflow -> exp kept fp32); normalize via tensor_scalar_mul -> fp16 attn.
  - attn^T via PE transpose (identity), AV: lhsT = v tiles, rhs = attn^T,
    nh pairs col-packed -> UT psum (128=(nh,a), 256=tau) -> ut fp16.
  - Residual: lhsT = Wres halves (a, e), rhs = strided ut selection, psum
    (64=e, 512 rows); ScalarE Relu+bias; VectorE 32x32 block transpose and
    a block-aware DMA writes the fp32 output in natural (b, s, f*64+e) order.
"""
import numpy as np

B, S, F, E, A, NH = 256, 8, 32, 64, 64, 2
NCORES = 8
BC = B // NCORES            # 32 batches per core
ROWS = BC * S               # 256 projection rows
CD = F * E                  # 2048 contraction dim
ND = A * F * NH             # 4096 projection cols
KTILES = CD // 128          # 16
TTILES = ND // 128          # 32 column tiles per weight
NB = BC * NH                # 64 attention batches per core

_NC_CACHE = None


def build_bass():
    import concourse.bacc as bacc
    import concourse.tile as tile
    from concourse import mybir
    from concourse.masks import make_identity

    f16 = mybir.dt.float16
    f32 = mybir.dt.float32
    Exp = mybir.ActivationFunctionType.Exp
    Relu = mybir.ActivationFunctionType.Relu

    nc = bacc.Bacc("TRN2", target_bir_lowering=False, debug=False)

    hs_d = nc.dram_tensor("hs", [BC, S, CD], f16, kind="ExternalInput")
    wq_d = nc.dram_tensor("wq", [CD, ND], f16, kind="ExternalInput")
    wk_d = nc.dram_tensor("wk", [CD, ND], f16, kind="ExternalInput")
    wv_d = nc.dram_tensor("wv", [E, 2 * A], f16, kind="ExternalInput")
    wres_d = nc.dram_tensor("wres", [2 * A, E], f16, kind="ExternalInput")
    bias_d = nc.dram_tensor("bias", [E, 1], f32, kind="ExternalInput")
    out_d = nc.dram_tensor("out", [BC, S, CD], f32, kind="ExternalOutput")

    with tile.TileContext(nc) as tc:
        from contextlib import ExitStack
        with ExitStack() as ctx:
            singles = ctx.enter_context(tc.tile_pool(name="singles", bufs=1))

            # ---- constants / persistent tiles ----
            ident = singles.tile([128, 128], f16)
            make_identity(nc, ident)

            hsT = singles.tile([128, KTILES, ROWS], f16)      # Hs^T per k-tile
            hs2d = hs_d[:].rearrange("b s c -> (b s) c")
            for kt in range(KTILES):
                nc.sync.dma_start_transpose(
                    hsT[:, kt, :], hs2d[:, kt * 128:(kt + 1) * 128])

            wv_sb = singles.tile([128, 2 * A], f16)
            nc.sync.dma_start(wv_sb[0:64, :], wv_d[:])
            nc.sync.dma_start(wv_sb[64:128, :], wv_d[:])

            wres_sb = singles.tile([128, 2, E], f16)
            for half in range(2):
                for jh in range(2):
                    nc.sync.dma_start(
                        wres_sb[half * 64:(half + 1) * 64, jh, :],
                        wres_d[jh * 64:(jh + 1) * 64, :])

            bias_sb = singles.tile([E, 1], f32)
            nc.sync.dma_start(bias_sb[:, :], bias_d[:])

            qt = singles.tile([64, BC, NH, 256], f16)
            kt_ = singles.tile([64, BC, NH, 256], f16)
            stq = singles.tile([128, BC, NH, 4, F], f16)      # jh=1 staging
            stk = singles.tile([128, BC, NH, 4, F], f16)
            v_all = singles.tile([128, NB, 2, A], f16)
            denom = singles.tile([128, 2 * NB], f32)
            recip = singles.tile([128, 2 * NB], f32)
            ut = singles.tile([128, BC, 2, 128], f16)         # (nh,a) x (b, jh, sp*32+f)

            # ---- v projection (independent of Wq/Wk stream; scheduled early) ----
            with tc.tile_pool(name="vhs", bufs=3) as vhs_pool, \
                 tc.tile_pool(name="vps", bufs=4, space="PSUM") as vps_pool:
                for bpair in range(0, NB, 2):
                    vhsT = vhs_pool.tile([128, 128], f16)
                    for pi in range(2):
                        b, nh = divmod(bpair + pi, NH)
                        nc.sync.dma_start_transpose(
                            vhsT[pi * 64:(pi + 1) * 64, :],
                            hs_d[b, nh * 4:(nh + 1) * 4, :]
                            .rearrange("s (f e) -> (s f) e", e=E))
                    vps = [vps_pool.tile([128, 2 * A], f32, name=f"vps{i}",
                                         tag=f"vps{i}")
                           for i in range(2)]
                    for pi in range(2):
                        nc.tensor.matmul(
                            vps[pi][:, :],
                            lhsT=vhsT[pi * 64:(pi + 1) * 64, :],
                            rhs=wv_sb[pi * 64:(pi + 1) * 64, :],
                            start=True, stop=True,
                            tile_position=(pi * 64, 0))
                    for pi in range(2):
                        for h in range(2):
                            nc.vector.tensor_copy(
                                v_all[:, bpair + pi, h, :],
                                vps[pi][:, h * 64:(h + 1) * 64])

            # ---- Q/K projection + gathers ----
            with tc.tile_pool(name="wtile", bufs=3) as w_pool, \
                 tc.tile_pool(name="pp", bufs=2, space="PSUM") as pp_pool:
                for w_d, dest, stage, cp_eng in (
                        (wq_d, qt, stq, nc.scalar),
                        (wk_d, kt_, stk, nc.vector)):
                    for t in range(TTILES):
                        wt = w_pool.tile([128, KTILES, 128], f16)
                        nc.sync.dma_start(
                            wt[:, :, :],
                            w_d[:, t * 128:(t + 1) * 128]
                            .rearrange("(kt p) c -> p kt c", p=128))
                        pp = pp_pool.tile([128, ROWS], f32)
                        for kt in range(KTILES):
                            nc.tensor.matmul(
                                pp[:, :], lhsT=wt[:, kt, :], rhs=hsT[:, kt, :],
                                start=(kt == 0), stop=(kt == KTILES - 1))
                        ppv = pp.rearrange("p (b n sp) -> p b n sp", b=BC, n=NH)
                        # jh=0 half: direct (p -> p) into dest at tau offset f=t
                        dview = dest.rearrange(
                            "p b n (j sf) -> p b n j sf", j=2)[:, :, :, 0, :] \
                            .rearrange("p b n (sp f) -> p b n sp f", sp=4)
                        if cp_eng is nc.scalar:
                            cp_eng.copy(dview[:, :, :, :, t], ppv[0:64])
                        else:
                            cp_eng.tensor_copy(dview[:, :, :, :, t], ppv[0:64])
                        # jh=1 half: stage at partitions 64..127
                        sview = stage[64:128, :, :, :, t]
                        if cp_eng is nc.scalar:
                            cp_eng.copy(sview, ppv[64:128])
                        else:
                            cp_eng.tensor_copy(sview, ppv[64:128])
                    # partition shift 64..127 -> 0..63 via SBUF->SBUF DMA
                    dhalf = dest.rearrange("p b n (j sf) -> p b n j sf", j=2)
                    nc.sync.dma_start(
                        dhalf[:, :, :, 1, :].rearrange(
                            "p b n (sp f) -> p b n sp f", sp=4),
                        stage[64:128, :, :, :, :])

            # ---- attention ----
            with tc.tile_pool(name="zps", bufs=2, space="PSUM") as z_pool, \
                 tc.tile_pool(name="tps", bufs=2, space="PSUM") as t_pool, \
                 tc.tile_pool(name="aps", bufs=2, space="PSUM") as a_pool, \
                 tc.tile_pool(name="expz", bufs=3) as e_pool, \
                 tc.tile_pool(name="attn", bufs=2) as at_pool, \
                 tc.tile_pool(name="attnT", bufs=2) as att_pool:
                for b in range(BC):
                    av = a_pool.tile([128, 256], f32)
                    for nh in range(NH):
                        bn = b * NH + nh
                        zt = z_pool.tile([128, 2, 256], f32)
                        attn = at_pool.tile([128, 2, 256], f16)
                        for jh in range(2):
                            nc.tensor.matmul(
                                zt[:, jh, :],
                                lhsT=qt[:, b, nh, jh * 128:(jh + 1) * 128],
                                rhs=kt_[:, b, nh, :], start=True, stop=True)
                        ez = e_pool.tile([128, 2, 256], f32)
                        for jh in range(2):
                            nc.scalar.activation(
                                ez[:, jh, :], zt[:, jh, :], Exp,
                                accum_out=denom[:, 2 * bn + jh:2 * bn + jh + 1])
                        nc.vector.reciprocal(
                            recip[:, 2 * bn:2 * bn + 2],
                            denom[:, 2 * bn:2 * bn + 2])
                        for jh in range(2):
                            nc.vector.tensor_scalar_mul(
                                attn[:, jh, :], ez[:, jh, :],
                                recip[:, 2 * bn + jh:2 * bn + jh + 1])
                        atT = att_pool.tile([128, 2, 256], f16)
                        for kk in range(2):
                            for jh in range(2):
                                tp = t_pool.tile([128, 128], f16)
                                nc.tensor.transpose(
                                    tp[:, :],
                                    attn[:, jh, kk * 128:(kk + 1) * 128],
                                    ident[:, :])
                                eng = nc.vector if jh == 0 else nc.scalar
                                if eng is nc.scalar:
                                    eng.copy(
                                        atT[:, kk, jh * 128:(jh + 1) * 128], tp)
                                else:
                                    eng.tensor_copy(
                                        atT[:, kk, jh * 128:(jh + 1) * 128], tp)
                        for kk in range(2):
                            nc.tensor.matmul(
                                av[nh * 64:(nh + 1) * 64, :],
                                lhsT=v_all[:, bn, kk, :],
                                rhs=atT[:, kk, :],
                                start=(kk == 0), stop=(kk == 1),
                                tile_position=(0, nh * 64))
                    nc.vector.tensor_copy(
                        ut[:, b, :, :].rearrange("p a b -> p (a b)"),
                        av[:, :])

            # ---- residual projection + relu + output ----
            outv = out_d[:].rearrange(
                "b (n sp) (f P j) -> f b n sp P j", n=NH, f=F, P=2)
            with tc.tile_pool(name="rps", bufs=2, space="PSUM") as r_pool, \
                 tc.tile_pool(name="fo", bufs=3) as f_pool, \
                 tc.tile_pool(name="ft", bufs=3) as ft_pool:
                for nh in range(NH):
                    for bg in range(BC // 4):
                        rp = r_pool.tile([64, 512], f32)
                        for jh in range(2):
                            nc.tensor.matmul(
                                rp[:, :],
                                lhsT=wres_sb[nh * 64:(nh + 1) * 64, jh, :],
                                rhs=ut[nh * 64:(nh + 1) * 64,
                                       bg * 4:(bg + 1) * 4, jh, :],
                                start=(jh == 0), stop=(jh == 1),
                                tile_position=(nh * 64, 0))
                        fo = f_pool.tile([64, 512], f32)
                        nc.scalar.activation(fo[:, :], rp[:, :], Relu,
                                             bias=bias_sb[:, :])
                        ft = ft_pool.tile([64, 512], f32)
                        nc.vector.transpose(ft[:, :], fo[:, :])
                        ftv = ft.rearrange("p (b sp j) -> p b sp j", b=4, sp=4)
                        for P in range(2):
                            for bi in range(4):
                                nc.sync.dma_start(
                                    outv[:, bg * 4 + bi, nh, :, P, :],
                                    ftv[P * 32:(P + 1) * 32, bi, :, :])
    nc.compile()
    return nc


def _get_nc():
    global _NC_CACHE
    if _NC_CACHE is None:
        _NC_CACHE = build_bass()
    return _NC_CACHE


def make_in_maps(Hs, Wq, Wk, Wv, Wres_w, Wres_b):
    wq16 = Wq.astype(np.float16)
    wk16 = Wk.astype(np.float16)
    wv16 = Wv.astype(np.float16)
    wres16 = Wres_w.astype(np.float16)
    bias = Wres_b.astype(np.float32).reshape(E, 1)
    hs16 = Hs.astype(np.float16)
    return [{
        "hs": np.ascontiguousarray(hs16[c * BC:(c + 1) * BC]),
        "wq": wq16, "wk": wk16, "wv": wv16, "wres": wres16, "bias": bias,
    } for c in range(NCORES)]


def kernel(Hs, Wq, Wk, Wv, Wres_w, Wres_b):
    from concourse.bass_utils import run_bass_kernel_spmd
    nc = _get_nc()
    in_maps = make_in_maps(Hs, Wq, Wk, Wv, Wres_w, Wres_b)
    res = run_bass_kernel_spmd(nc, in_maps, list(range(NCORES)))
    out = np.concatenate(
        [np.asarray(res.results[c]["out"]) for c in range(NCORES)], axis=0)
    return out.astype(np.float32)


if __name__ == "__main__":
    nc = build_bass()
    print("built OK; instructions:",
          sum(len(bb.instructions) for fn in nc.m.functions
              for bb in fn.blocks))


# revision 12
# speedup vs baseline: 2.7450x; 2.7450x over previous
"""Trainium2 Bass kernel for nn_MultiHeadSelfAttention_88725434400988.

Self-contained: accepts FULL inputs, shards batch B=256 over 8 NeuronCores
(32 per core), runs one SPMD Bass program, gathers the FULL output.

Per-core algorithm (B_CORE=32, S=8, F=32, E=64, A=64, NH=2):
  - Hs, Wq, Wk, Wv, Wres cast to fp16 on host (PE matmuls run 1 cyc/row,
    fp32 PSUM accumulation; end-to-end error vs fp32 reference ~3e-3 absmax).
  - All transposed/tiled operands are pre-laid-out on HOST into HBM buffers
    whose per-partition bytes are contiguous, so every big DMA moves
    multi-KB descriptor runs (transposing/strided DMAs measured 650us of
    sync-engine serialization in v1).
  - QK projection: lhsT = 128-col tiles of W, rhs = Hs^T; psum
    (128=(jh,a), 256=(b,s)).
  - Working order for attention rows/cols: p = jh*128 + sp*32 + f where the
    original index is sp*64 + 2f + jh (jh == partition half of psum).
    q/k gathered into qt/kt (64=a, jh, b, nh, 128) fp16; the jh=1 psum half
    is staged and moved across partitions with one SBUF->SBUF DMA.
  - v: lhsT = host-transposed Hs rows (e, 128 rows) per (b,nh) pair;
    out (128 rows=(sp,f), 128=(h,a)) -> v_all (128=sigma tile, bn, 2, 64).
  - Z = qt_chunk.T @ kt (per jh chunk); exp on ScalarE with fused accum_out
    denominators (fp32; max |logit| ~49 so no max subtraction, fp16 would
    overflow -> exp kept fp32); normalize via tensor_scalar_mul -> fp16 attn.
  - attn^T via PE transpose (identity), AV: lhsT = v tiles, rhs = attn^T,
    nh pairs col-packed -> UT psum (128=(nh,a), 256=tau) -> ut fp16.
  - Residual: lhsT = Wres halves (a, e), rhs = strided ut selection, psum
    (64=e, 512 rows); ScalarE Relu+bias -> SBUF -> contiguous DMA into a
    (64, 8192) staging output; host un-permutes to (B, S, F*E) fp32.
"""
import numpy as np

B, S, F, E, A, NH = 256, 8, 32, 64, 64, 2
NCORES = 8
BC = B // NCORES            # 32 batches per core
ROWS = BC * S               # 256 projection rows
CD = F * E                  # 2048 contraction dim
ND = A * F * NH             # 4096 projection cols
KTILES = CD // 128          # 16
TTILES = ND // 128          # 32 column tiles per weight
NB = BC * NH                # 64 attention batches per core
WCHUNK = 4                  # weight tiles per DMA (4 * 512KB = 2MB)

_NC_CACHE = None


def build_bass():
    import concourse.bacc as bacc
    import concourse.tile as tile
    from concourse import mybir
    from concourse.masks import make_identity

    f16 = mybir.dt.float16
    f32 = mybir.dt.float32
    Exp = mybir.ActivationFunctionType.Exp
    Relu = mybir.ActivationFunctionType.Relu

    nc = bacc.Bacc("TRN2", target_bir_lowering=False, debug=False)

    # host-prepped layouts (see make_in_maps)
    hst_d = nc.dram_tensor("hst", [128, KTILES, ROWS], f16, kind="ExternalInput")
    hsv_d = nc.dram_tensor("hsv", [128, NB // 2, 128], f16, kind="ExternalInput")
    wq_d = nc.dram_tensor("wq", [128, TTILES, KTILES * 128], f16,
                          kind="ExternalInput")
    wk_d = nc.dram_tensor("wk", [128, TTILES, KTILES * 128], f16,
                          kind="ExternalInput")
    wv_d = nc.dram_tensor("wv", [E, 2 * A], f16, kind="ExternalInput")
    wres_d = nc.dram_tensor("wres", [2 * A, E], f16, kind="ExternalInput")
    bias_d = nc.dram_tensor("bias", [E, 1], f32, kind="ExternalInput")
    out_d = nc.dram_tensor("out", [E, BC * S * F], f32, kind="ExternalOutput")

    with tile.TileContext(nc) as tc:
        from contextlib import ExitStack
        with ExitStack() as ctx:
            singles = ctx.enter_context(tc.tile_pool(name="singles", bufs=1))

            # ---- constants / persistent tiles ----
            ident = singles.tile([128, 128], f16)
            make_identity(nc, ident)

            hsT = singles.tile([128, KTILES, ROWS], f16)
            nc.gpsimd.dma_start(hsT[:, :, :], hst_d[:])
            hsv = singles.tile([128, NB // 2, 128], f16)
            nc.gpsimd.dma_start(hsv[:, :, :], hsv_d[:])

            wv_sb = singles.tile([128, 2 * A], f16)
            nc.sync.dma_start(wv_sb[0:64, :], wv_d[:])
            nc.sync.dma_start(wv_sb[64:128, :], wv_d[:])

            wres_sb = singles.tile([128, 2, E], f16)
            for half in range(2):
                for jh in range(2):
                    nc.sync.dma_start(
                        wres_sb[half * 64:(half + 1) * 64, jh, :],
                        wres_d[jh * 64:(jh + 1) * 64, :])

            bias_sb = singles.tile([E, 1], f32)
            nc.sync.dma_start(bias_sb[:, :], bias_d[:])

            qt = singles.tile([64, 2, BC, NH, 128], f16)
            kt_ = singles.tile([64, 2, BC, NH, 128], f16)
            stq = singles.tile([128, BC, NH, 4, F], f16)      # jh=1 staging
            stk = singles.tile([128, BC, NH, 4, F], f16)
            v_all = singles.tile([128, NB, 2, A], f16)
            denom = singles.tile([128, 2 * NB], f32)
            recip = singles.tile([128, 2 * NB], f32)
            ut = singles.tile([128, BC, 2, 128], f16)         # (nh,a) x (b, jh, sp*32+f)

            # ---- v projection (independent of Wq/Wk stream; scheduled early) ----
            with tc.tile_pool(name="vps", bufs=4, space="PSUM") as vps_pool:
                for bpair in range(0, NB, 2):
                    vps = [vps_pool.tile([128, 2 * A], f32, name=f"vps{i}",
                                         tag=f"vps{i}")
                           for i in range(2)]
                    for pi in range(2):
                        nc.tensor.matmul(
                            vps[pi][:, :],
                            lhsT=hsv[pi * 64:(pi + 1) * 64, bpair // 2, :],
                            rhs=wv_sb[pi * 64:(pi + 1) * 64, :],
                            start=True, stop=True,
                            tile_position=(pi * 64, 0))
                    for pi in range(2):
                        for h in range(2):
                            nc.vector.tensor_copy(
                                v_all[:, bpair + pi, h, :],
                                vps[pi][:, h * 64:(h + 1) * 64])

            # ---- Q/K projection + gathers ----
            with tc.tile_pool(name="wtile", bufs=2) as w_pool, \
                 tc.tile_pool(name="pp", bufs=2, space="PSUM") as pp_pool:
                for w_d, dest, stage, cp_eng, dma_eng in (
                        (wq_d, qt, stq, nc.scalar, nc.sync),
                        (wk_d, kt_, stk, nc.vector, nc.gpsimd)):
                    for tg in range(TTILES // WCHUNK):
                        wt = w_pool.tile([128, WCHUNK, KTILES, 128], f16,
                                         name="wt", tag="wt")
                        dma_eng.dma_start(
                            wt[:, :, :, :],
                            w_d[:, tg * WCHUNK:(tg + 1) * WCHUNK, :]
                            .rearrange("p t (kt c) -> p t kt c", c=128))
                        for ti in range(WCHUNK):
                            t = tg * WCHUNK + ti
                            pp = pp_pool.tile([128, ROWS], f32)
                            for kt in range(KTILES):
                                nc.tensor.matmul(
                                    pp[:, :], lhsT=wt[:, ti, kt, :],
                                    rhs=hsT[:, kt, :],
                                    start=(kt == 0), stop=(kt == KTILES - 1))
                            ppv = pp.rearrange("p (b n sp) -> p b n sp",
                                               b=BC, n=NH)
                            # jh=0 half: direct (p -> p) at tau offset f=t
                            dview = dest[:, 0, :, :, :].rearrange(
                                "p b n (sp f) -> p b n sp f", sp=4)
                            if cp_eng is nc.scalar:
                                cp_eng.copy(dview[:, :, :, :, t], ppv[0:64])
                            else:
                                cp_eng.tensor_copy(dview[:, :, :, :, t],
                                                   ppv[0:64])
                            # jh=1 half: stage at partitions 64..127
                            sview = stage[64:128, :, :, :, t]
                            if cp_eng is nc.scalar:
                                cp_eng.copy(sview, ppv[64:128])
                            else:
                                cp_eng.tensor_copy(sview, ppv[64:128])
                    # partition shift 64..127 -> 0..63 via SBUF->SBUF DMA
                    nc.gpsimd.dma_start(
                        dest[:, 1, :, :, :],
                        stage[64:128, :, :, :, :]
                        .rearrange("p b n sp f -> p b n (sp f)"))

            # ---- attention ----
            with tc.tile_pool(name="zps", bufs=2, space="PSUM") as z_pool, \
                 tc.tile_pool(name="tps", bufs=2, space="PSUM") as t_pool, \
                 tc.tile_pool(name="aps", bufs=2, space="PSUM") as a_pool, \
                 tc.tile_pool(name="expz", bufs=3) as e_pool, \
                 tc.tile_pool(name="attn", bufs=2) as at_pool, \
                 tc.tile_pool(name="attnT", bufs=2) as att_pool:
                for b in range(BC):
                    av = a_pool.tile([128, 256], f32)
                    for nh in range(NH):
                        bn = b * NH + nh
                        zt = z_pool.tile([128, 2, 256], f32)
                        attn = at_pool.tile([128, 2, 256], f16)
                        for jh in range(2):
                            nc.tensor.matmul(
                                zt[:, jh, :],
                                lhsT=qt[:, jh, b, nh, :],
                                rhs=kt_[:, :, b, nh, :],
                                start=True, stop=True)
                        ez = e_pool.tile([128, 2, 256], f32)
                        for jh in range(2):
                            nc.scalar.activation(
                                ez[:, jh, :], zt[:, jh, :], Exp,
                                accum_out=denom[:, 2 * bn + jh:2 * bn + jh + 1])
                        nc.vector.reciprocal(
                            recip[:, 2 * bn:2 * bn + 2],
                            denom[:, 2 * bn:2 * bn + 2])
                        for jh in range(2):
                            nc.vector.tensor_scalar_mul(
                                attn[:, jh, :], ez[:, jh, :],
                                recip[:, 2 * bn + jh:2 * bn + jh + 1])
                        atT = att_pool.tile([128, 2, 256], f16)
                        for kk in range(2):
                            for jh in range(2):
                                tp = t_pool.tile([128, 128], f16)
                                nc.tensor.transpose(
                                    tp[:, :],
                                    attn[:, jh, kk * 128:(kk + 1) * 128],
                                    ident[:, :])
                                if jh == 0:
                                    nc.vector.tensor_copy(
                                        atT[:, kk, jh * 128:(jh + 1) * 128], tp)
                                else:
                                    nc.scalar.copy(
                                        atT[:, kk, jh * 128:(jh + 1) * 128], tp)
                        for kk in range(2):
                            nc.tensor.matmul(
                                av[nh * 64:(nh + 1) * 64, :],
                                lhsT=v_all[:, bn, kk, :],
                                rhs=atT[:, kk, :],
                                start=(kk == 0), stop=(kk == 1),
                                tile_position=(0, nh * 64))
                    nc.vector.tensor_copy(
                        ut[:, b, :, :].rearrange("p a b -> p (a b)"),
                        av[:, :])

            # ---- residual projection + relu + output (packed layout) ----
            with tc.tile_pool(name="rps", bufs=2, space="PSUM") as r_pool, \
                 tc.tile_pool(name="fo", bufs=3) as f_pool:
                for nh in range(NH):
                    for bg in range(BC // 4):
                        rp = r_pool.tile([64, 512], f32)
                        for jh in range(2):
                            nc.tensor.matmul(
                                rp[:, :],
                                lhsT=wres_sb[nh * 64:(nh + 1) * 64, jh, :],
                                rhs=ut[nh * 64:(nh + 1) * 64,
                                       bg * 4:(bg + 1) * 4, jh, :],
                                start=(jh == 0), stop=(jh == 1),
                                tile_position=(nh * 64, 0))
                        fo = f_pool.tile([64, 512], f32)
                        nc.scalar.activation(fo[:, :], rp[:, :], Relu,
                                             bias=bias_sb[:, :])
                        nc.sync.dma_start(
                            out_d[:, (nh * (BC // 4) + bg) * 512:
                                  (nh * (BC // 4) + bg + 1) * 512],
                            fo[:, :])
    nc.compile()
    return nc


def _get_nc():
    global _NC_CACHE
    if _NC_CACHE is None:
        _NC_CACHE = build_bass()
    return _NC_CACHE


def _prep_weight(W):
    # (CD, ND) -> (128, TTILES, KTILES*128): [p, t, kt*128+j] = W[kt*128+p, t*128+j]
    return np.ascontiguousarray(
        W.astype(np.float16).reshape(KTILES, 128, TTILES, 128)
        .transpose(1, 2, 0, 3).reshape(128, TTILES, KTILES * 128))


def make_in_maps(Hs, Wq, Wk, Wv, Wres_w, Wres_b):
    wq16 = _prep_weight(Wq)
    wk16 = _prep_weight(Wk)
    wv16 = Wv.astype(np.float16)
    wres16 = Wres_w.astype(np.float16)
    bias = Wres_b.astype(np.float32).reshape(E, 1)
    hs16 = Hs.astype(np.float16)
    maps = []
    for c in range(NCORES):
        sh = hs16[c * BC:(c + 1) * BC]                      # (BC, S, CD)
        hs2d = sh.reshape(ROWS, CD)
        hst = np.ascontiguousarray(
            hs2d.reshape(ROWS, KTILES, 128).transpose(2, 1, 0))
        # hsv[pi*64+e, q, r] = Hs[b, nh*4+sp, f, e]; bn = 2q+pi = b*NH+nh
        arr = sh.reshape(NB, 128, E)                        # (bn, (sp,f), e)
        hsv = np.ascontiguousarray(
            arr.reshape(NB // 2, 2, 128, E).transpose(1, 3, 0, 2)
            .reshape(128, NB // 2, 128))
        maps.append({
            "hst": hst, "hsv": hsv,
            "wq": wq16, "wk": wk16, "wv": wv16, "wres": wres16, "bias": bias,
        })
    return maps


def _unpack_out(o):
    # o: (E, BC*S*F) = (e, nh, bg, b4, sp, f) -> (BC, S, F*E)
    o = o.reshape(E, NH, BC // 4, 4, 4, F)
    return np.ascontiguousarray(
        o.transpose(2, 3, 1, 4, 5, 0)).reshape(BC, S, F * E)


def kernel(Hs, Wq, Wk, Wv, Wres_w, Wres_b):
    from concourse.bass_utils import run_bass_kernel_spmd
    nc = _get_nc()
    in_maps = make_in_maps(Hs, Wq, Wk, Wv, Wres_w, Wres_b)
    res = run_bass_kernel_spmd(nc, in_maps, list(range(NCORES)))
    out = np.concatenate(
        [_unpack_out(np.asarray(res.results[c]["out"]))
         for c in range(NCORES)], axis=0)
    return out.astype(np.float32)


if __name__ == "__main__":
    nc = build_bass()
    print("built OK; instructions:",
          sum(len(bb.instructions) for fn in nc.m.functions
              for bb in fn.blocks))


# revision 19
# speedup vs baseline: 2.9515x; 1.0753x over previous
"""Trainium2 Bass kernel for nn_MultiHeadSelfAttention_88725434400988.

Self-contained: accepts FULL inputs, shards batch B=256 over 8 NeuronCores
(32 per core), runs one SPMD Bass program, gathers the FULL output.

Per-core algorithm (B_CORE=32, S=8, F=32, E=64, A=64, NH=2):
  - Hs, Wq, Wk, Wv, Wres cast to fp16 on host (PE matmuls run 1 cyc/row,
    fp32 PSUM accumulation; end-to-end error vs fp32 reference ~3.6e-3
    absmax / 1.9e-3 l2-relative).
  - All transposed/tiled operands are pre-laid-out on HOST into HBM buffers
    whose per-partition bytes are contiguous, so every big DMA moves
    multi-KB descriptor runs.
  - Working label order for attention rows/cols: p = jh*128 + f*4 + sp,
    where the original index is sp*64 + 2f + jh (jh == psum partition half
    of the projection output; the order makes gather copies 3-dim strided
    with 32B-contiguous runs).
  - QK projection: lhsT = 128-col tiles of W, rhs = Hs^T; psum groups of 4
    tiles (128=(jh,a), 4, 256=(b,s)); one batched copy per (group, jh).
    The jh=1 halves are staged and partition-shifted with one SBUF->SBUF
    DMA (engines cannot cross partitions; DMA can).
  - v: lhsT = host-transposed Hs rows (e, 128 rows) per (b,nh) pair ->
    v_all bf16 (128=sigma tile, bn, 2, 64).
  - Attention is TRANSPOSE-FREE: Z^T = k_chunk.T @ q (sigma on partition),
    exp on ScalarE -> bf16 (fp32 range, no overflow at |z|<=49; fp16 would
    overflow), denominators via ones-vector matmul (free), reciprocal on
    VectorE, broadcast across partitions with a stride-0 DMA, and the
    normalization multiply is fused into the UT psum evacuation.
  - AV: lhsT = v tiles (bf16), rhs = exp(Z^T) (bf16), nh pairs col-packed
    -> UT psum (128=(nh,a), 256=tau) -> ut fp16 (x recip).
  - Residual: lhsT = Wres halves (a, e), rhs = strided ut selection, psum
    (64=e, 512 rows); ScalarE Relu+bias -> SBUF -> contiguous DMA into a
    (64, 8192) staging output; host un-permutes to (B, S, F*E) fp32.
"""
import numpy as np

B, S, F, E, A, NH = 256, 8, 32, 64, 64, 2
NCORES = 8
BC = B // NCORES            # 32 batches per core
ROWS = BC * S               # 256 projection rows
CD = F * E                  # 2048 contraction dim
ND = A * F * NH             # 4096 projection cols
KTILES = CD // 128          # 16
TTILES = ND // 128          # 32 column tiles per weight
NB = BC * NH                # 64 attention batches per core
WCHUNK = 4                  # weight tiles per DMA
GT = 4                      # projection tiles batched per psum/copy group

_NC_CACHE = None


def build_bass():
    import concourse.bacc as bacc
    import concourse.tile as tile
    from concourse import mybir

    f16 = mybir.dt.float16
    bf16 = mybir.dt.bfloat16
    f32 = mybir.dt.float32
    Exp = mybir.ActivationFunctionType.Exp
    Relu = mybir.ActivationFunctionType.Relu

    nc = bacc.Bacc("TRN2", target_bir_lowering=False, debug=False)

    # host-prepped layouts (see make_in_maps)
    hst_d = nc.dram_tensor("hst", [128, KTILES, ROWS], f16, kind="ExternalInput")
    hsv_d = nc.dram_tensor("hsv", [128, NB // 2, 128], f16, kind="ExternalInput")
    wq_d = nc.dram_tensor("wq", [128, TTILES, KTILES * 128], f16,
                          kind="ExternalInput")
    wk_d = nc.dram_tensor("wk", [128, TTILES, KTILES * 128], f16,
                          kind="ExternalInput")
    wv_d = nc.dram_tensor("wv", [E, 2 * A], f16, kind="ExternalInput")
    wres_d = nc.dram_tensor("wres", [2 * A, E], f16, kind="ExternalInput")
    bias_d = nc.dram_tensor("bias", [E, 1], f32, kind="ExternalInput")
    out_d = nc.dram_tensor("out", [E, BC * S * F], f32, kind="ExternalOutput")

    with tile.TileContext(nc) as tc:
        from contextlib import ExitStack
        with ExitStack() as ctx:
            singles = ctx.enter_context(tc.tile_pool(name="singles", bufs=1))

            # ---- constants / persistent tiles ----
            ones_bf = singles.tile([128, 1], bf16)
            nc.vector.memset(ones_bf, 1.0)

            hsT = singles.tile([128, KTILES, ROWS], f16)
            nc.gpsimd.dma_start(hsT[:, :, :], hst_d[:])
            hsv = singles.tile([128, NB // 2, 128], f16)
            nc.gpsimd.dma_start(hsv[:, :, :], hsv_d[:])

            wv_sb = singles.tile([128, 2 * A], f16)
            nc.sync.dma_start(wv_sb[0:64, :], wv_d[:])
            nc.sync.dma_start(wv_sb[64:128, :], wv_d[:])

            wres_sb = singles.tile([128, 2, E], f16)
            for half in range(2):
                for jh in range(2):
                    nc.sync.dma_start(
                        wres_sb[half * 64:(half + 1) * 64, jh, :],
                        wres_d[jh * 64:(jh + 1) * 64, :])

            bias_sb = singles.tile([E, 1], f32)
            nc.sync.dma_start(bias_sb[:, :], bias_d[:])

            qt = singles.tile([64, 2, BC, NH, 128], f16)
            kt_ = singles.tile([64, 2, BC, NH, 128], f16)
            v_all = singles.tile([128, NB, 2, A], bf16)
            ut = singles.tile([128, BC, 2, 128], f16)  # (nh,a) x (b, jh, f*4+sp)

            # ---- v projection (independent of Wq/Wk stream) ----
            with tc.tile_pool(name="vps", bufs=2, space="PSUM") as vps_pool:
                for bpair in range(0, NB, 2):
                    vps = [vps_pool.tile([128, 2 * A], f32, name=f"vps{i}",
                                         tag=f"vps{i}")
                           for i in range(2)]
                    for pi in range(2):
                        nc.tensor.matmul(
                            vps[pi][:, :],
                            lhsT=hsv[pi * 64:(pi + 1) * 64, bpair // 2, :],
                            rhs=wv_sb[pi * 64:(pi + 1) * 64, :],
                            start=True, stop=True,
                            tile_position=(pi * 64, 0))
                    for pi in range(2):
                        nc.vector.tensor_copy(
                            v_all[:, bpair + pi, :, :], vps[pi][:, :])

            # ---- Q/K projection + batched gathers ----
            with tc.tile_pool(name="wtile", bufs=2) as w_pool, \
                 tc.tile_pool(name="stage", bufs=1) as st_pool, \
                 tc.tile_pool(name="pp", bufs=2, space="PSUM") as pp_pool:
                for w_d, dest, cp_eng, dma_eng in (
                        (wq_d, qt, nc.scalar, nc.sync),
                        (wk_d, kt_, nc.vector, nc.gpsimd)):
                    stage = st_pool.tile([128, BC, NH, 128], f16,
                                         name="stage", tag="stage")
                    for tg in range(TTILES // WCHUNK):
                        wt = w_pool.tile([128, WCHUNK, KTILES, 128], f16,
                                         name="wt", tag="wt")
                        dma_eng.dma_start(
                            wt[:, :, :, :],
                            w_d[:, tg * WCHUNK:(tg + 1) * WCHUNK, :]
                            .rearrange("p t (kt c) -> p t kt c", c=128))
                        for gi in range(WCHUNK // GT):
                            pp = pp_pool.tile([128, GT, ROWS], f32)
                            for ti in range(GT):
                                t = tg * WCHUNK + gi * GT + ti
                                for kt in range(KTILES):
                                    nc.tensor.matmul(
                                        pp[:, ti, :],
                                        lhsT=wt[:, gi * GT + ti, kt, :],
                                        rhs=hsT[:, kt, :],
                                        start=(kt == 0),
                                        stop=(kt == KTILES - 1))
                            # psum free (ti, b, nh, sp) -> iterate (bn, ti, sp)
                            src = pp.rearrange(
                                "p ti (b n sp) -> p (b n) ti sp", n=NH, sp=4)
                            t0 = tg * WCHUNK + gi * GT
                            dv = dest[:, 0, :, :, :].rearrange(
                                "p b n (f sp) -> p (b n) f sp", sp=4)
                            sv = stage[:, :, :, :].rearrange(
                                "p b n (f sp) -> p (b n) f sp", sp=4)
                            if cp_eng is nc.scalar:
                                cp_eng.copy(
                                    dv[:, :, t0:t0 + GT, :], src[0:64])
                                cp_eng.copy(
                                    sv[64:128, :, t0:t0 + GT, :], src[64:128])
                            else:
                                cp_eng.tensor_copy(
                                    dv[:, :, t0:t0 + GT, :], src[0:64])
                                cp_eng.tensor_copy(
                                    sv[64:128, :, t0:t0 + GT, :], src[64:128])
                    # partition shift 64..127 -> 0..63 via SBUF->SBUF DMA
                    nc.gpsimd.dma_start(
                        dest[:, 1, :, :, :],
                        stage[64:128, :, :, :])

            # ---- attention (transpose-free, Z^T layout) ----
            with tc.tile_pool(name="zps", bufs=2, space="PSUM") as z_pool, \
                 tc.tile_pool(name="dps", bufs=2, space="PSUM") as d_pool, \
                 tc.tile_pool(name="aps", bufs=2, space="PSUM") as a_pool, \
                 tc.tile_pool(name="expz", bufs=3) as e_pool, \
                 tc.tile_pool(name="recs", bufs=2) as rc_pool, \
                 tc.tile_pool(name="reps", bufs=2) as rp_pool:
                for b in range(BC):
                    av = a_pool.tile([128, 256], f32)
                    denp = d_pool.tile([1, 2, 256], f32)
                    ezs = []
                    for nh in range(NH):
                        bn = b * NH + nh
                        zt = z_pool.tile([128, 2, 256], f32, name="zt", tag="zt")
                        for h in range(2):
                            nc.tensor.matmul(
                                zt[:, h, :],
                                lhsT=kt_[:, h, b, nh, :],
                                rhs=qt[:, :, b, nh, :],
                                start=True, stop=True)
                        ez = e_pool.tile([128, 2, 256], bf16, name="ez", tag="ez")
                        ezs.append(ez)
                        for h in range(2):
                            nc.scalar.activation(ez[:, h, :], zt[:, h, :], Exp)
                        for h in range(2):
                            nc.tensor.matmul(
                                denp[:, nh, :], lhsT=ones_bf[:, :],
                                rhs=ez[:, h, :],
                                start=(h == 0), stop=(h == 1))
                    rec = rc_pool.tile([1, 2, 256], f32)
                    nc.vector.reciprocal(
                        rec[:, :, :].rearrange("p a b -> p (a b)"),
                        denp[:, :, :].rearrange("p a b -> p (a b)"))
                    rep = rp_pool.tile([128, 2, 256], f32)
                    for nh in range(NH):
                        nc.gpsimd.partition_broadcast(
                            rep[:, nh, :], rec[:, nh, :])
                    for nh in range(NH):
                        bn = b * NH + nh
                        for kk in range(2):
                            nc.tensor.matmul(
                                av[nh * 64:(nh + 1) * 64, :],
                                lhsT=v_all[:, bn, kk, :],
                                rhs=ezs[nh][:, kk, :],
                                start=(kk == 0), stop=(kk == 1),
                                tile_position=(0, nh * 64))
                    for nh in range(NH):
                        nc.vector.tensor_mul(
                            ut[nh * 64:(nh + 1) * 64, b, :, :]
                            .rearrange("p a b -> p (a b)"),
                            av[nh * 64:(nh + 1) * 64, :],
                            rep[nh * 64:(nh + 1) * 64, nh, :])

            # ---- residual projection + relu + output (packed layout) ----
            with tc.tile_pool(name="rps", bufs=2, space="PSUM") as r_pool, \
                 tc.tile_pool(name="fo", bufs=3) as f_pool:
                for nh in range(NH):
                    for bg in range(BC // 4):
                        rp = r_pool.tile([64, 512], f32)
                        for jh in range(2):
                            nc.tensor.matmul(
                                rp[:, :],
                                lhsT=wres_sb[nh * 64:(nh + 1) * 64, jh, :],
                                rhs=ut[nh * 64:(nh + 1) * 64,
                                       bg * 4:(bg + 1) * 4, jh, :],
                                start=(jh == 0), stop=(jh == 1),
                                tile_position=(nh * 64, 0))
                        fo = f_pool.tile([64, 512], f32)
                        nc.scalar.activation(fo[:, :], rp[:, :], Relu,
                                             bias=bias_sb[:, :])
                        nc.sync.dma_start(
                            out_d[:, (nh * (BC // 4) + bg) * 512:
                                  (nh * (BC // 4) + bg + 1) * 512],
                            fo[:, :])
    nc.compile()
    return nc


def _get_nc():
    global _NC_CACHE
    if _NC_CACHE is None:
        _NC_CACHE = build_bass()
    return _NC_CACHE


def _prep_weight(W):
    # (CD, ND) -> (128, TTILES, KTILES*128): [p, t, kt*128+j] = W[kt*128+p, t*128+j]
    return np.ascontiguousarray(
        W.astype(np.float16).reshape(KTILES, 128, TTILES, 128)
        .transpose(1, 2, 0, 3).reshape(128, TTILES, KTILES * 128))


def make_in_maps(Hs, Wq, Wk, Wv, Wres_w, Wres_b):
    wq16 = _prep_weight(Wq)
    wk16 = _prep_weight(Wk)
    wv16 = Wv.astype(np.float16)
    wres16 = Wres_w.astype(np.float16)
    bias = Wres_b.astype(np.float32).reshape(E, 1)
    hs16 = Hs.astype(np.float16)
    maps = []
    for c in range(NCORES):
        sh = hs16[c * BC:(c + 1) * BC]                      # (BC, S, CD)
        hs2d = sh.reshape(ROWS, CD)
        hst = np.ascontiguousarray(
            hs2d.reshape(ROWS, KTILES, 128).transpose(2, 1, 0))
        # v rows in sigma' order (f*4+sp):
        # hsv[pi*64+e, q, f*4+sp] = Hs[b, nh*4+sp, f, e]; bn = 2q+pi = b*NH+nh
        arr = sh.reshape(NB, 4, F, E).transpose(0, 2, 1, 3).reshape(NB, 128, E)
        hsv = np.ascontiguousarray(
            arr.reshape(NB // 2, 2, 128, E).transpose(1, 3, 0, 2)
            .reshape(128, NB // 2, 128))
        maps.append({
            "hst": hst, "hsv": hsv,
            "wq": wq16, "wk": wk16, "wv": wv16, "wres": wres16, "bias": bias,
        })
    return maps


def _unpack_out(o):
    # o: (E, BC*S*F) = (e, nh, bg, b4, f, sp) -> (BC, S, F*E)
    o = o.reshape(E, NH, BC // 4, 4, F, 4)
    return np.ascontiguousarray(
        o.transpose(2, 3, 1, 5, 4, 0)).reshape(BC, S, F * E)


def kernel(Hs, Wq, Wk, Wv, Wres_w, Wres_b):
    from concourse.bass_utils import run_bass_kernel_spmd
    nc = _get_nc()
    in_maps = make_in_maps(Hs, Wq, Wk, Wv, Wres_w, Wres_b)
    res = run_bass_kernel_spmd(nc, in_maps, list(range(NCORES)))
    out = np.concatenate(
        [_unpack_out(np.asarray(res.results[c]["out"]))
         for c in range(NCORES)], axis=0)
    return out.astype(np.float32)


if __name__ == "__main__":
    nc = build_bass()
    print("built OK; instructions:",
          sum(len(bb.instructions) for fn in nc.m.functions
              for bb in fn.blocks))
